# revision 13
# baseline (speedup 1.0000x reference)
"""DKVMN scatter_memory kernel for 8 Trainium2 NeuronCores.

Math: the reference scan only ever uses the (B, M, Dv) memory through
read @ Wf_r, so the whole recurrence collapses to a 32-dim linear
cumulative sum:

  S  = softmax(Eq @ Wa + ba)            (100 x 32)  per-vocab att rows
  cq = Eq @ Wf[:64] + bf                (100,)
  cv = Ev @ Wf[64:]                     (100,)
  w  = (2q + a) % 100
  pred[t,b] = cq[q[t,b]] + sum_{s<t} cv[w[s,b]] * <S[q[t,b]], S[q[s,b]]>

Per core (batch-sharded, Bs=128): the host precomputes a 120-row fp8
index encoding per token (pure index preprocessing; 0/1 exact in fp8):
rows 0:100 one-hot(q), rows 100:110 one-hot(w%10), rows 110:120
one-hot(w//10).  One 54-col matmul per batch element against a packed
table mcat = [S | cvt | ind | cq | pad] gathers the S-row, cq, and the
digit factors of cv[w] (cv[w] = sum_j 1{hi=j} * cv[10j+lo]).  The
cumsum over t is a strict-upper-triangular matmul.  Layout: t on
partitions, (b, m) on free dim.

Engine split per 32-batch pass: PE does 32 gather matmuls + 2 cumsum
matmuls; Scalar does two contiguous-dst PSUM->SBUF copies (A and the
digit/cq block); DVE does digit combine, A*C (reading C from PSUM) and
the segmented reduce; GpSimd does the cv broadcast-multiply.  One-hot
chunks stream in pass order, split across the two HWDGE rings.
"""
import functools
import numpy as np
import ml_dtypes

import concourse.bass as bass
import concourse.bacc as bacc
import concourse.mybir as mybir
from concourse import tile
from concourse.bass_utils import run_bass_kernel_spmd

T, B, M, DQ, DV, VOCAB = 128, 1024, 32, 64, 64, 100
NCORES = 8
BS = B // NCORES  # 128
N = T * BS        # tokens per core = 16384
R = 120           # one-hot rows: 100 q + 10 w-lo + 10 w-hi
GB = 32           # b per pass
NPASS = BS // GB  # 4
F32 = mybir.dt.float32
F16 = mybir.dt.float16
FP8 = mybir.dt.float8e4
AX = mybir.AxisListType
OP = mybir.AluOpType

# mcat column layout (53 used cols at stride 54):
#   0:32  S row      32:42 cvt (cv candidates given lo digit)
#   42:52 ind (1{hi=j})   52 cq   53 pad(0)
MC = 54

# packed-parameter column layout (f16 [128, PC])
_EQT, _EVT, _WAQ, _WFR, _BIA, _ONE, _US, _SKEL = (
    0, 100, 200, 233, 234, 268, 368, 496)
PC = _SKEL + MC  # 550

# merged per-pass workspace (f16), column offsets
_A = 0          # [32, 32]  gathered S rows, contiguous
_DGC = 1024     # [32, 22]  (cvt 0:10 | ind 10:20 | cq 20 | pad 21)
_CVP = 1728     # [32, 10]  cvt * ind
_CVW = 2048     # [32, 1]   cv[w]
_V = 2080       # [32, 32]  A * cv[w]
_AP = 3104      # [32, 32]  A * C
_O16 = 4128     # [32, 1]   reduced pred terms
WS = 4160


def _build():
    nc = bacc.Bacc("TRN2", num_devices=NCORES, debug=False, target_bir_lowering=False)
    d = {}
    d["pack"] = nc.dram_tensor("pack", [128, PC], F16, kind="ExternalInput").ap()
    d["ohall"] = nc.dram_tensor("ohall", [R, N], FP8, kind="ExternalInput").ap()
    preds = nc.dram_tensor("preds", [T, BS], F32, kind="ExternalOutput").ap()

    with tile.TileContext(nc) as tc:
        with (
            tc.tile_pool(name="sb", bufs=1) as sb,
            tc.tile_pool(name="oh", bufs=2) as ohp,
            tc.tile_pool(name="wk", bufs=2) as wk,
            tc.tile_pool(name="ps", bufs=3, space="PSUM") as ps,
        ):
            P = sb.tile([128, PC], F16)
            nc.scalar.dma_start(P[:], d["pack"][:])
            # mcat skeleton: zeros + I10 at rows 110:120, cols 42:52
            mcat = sb.tile([R, MC], F16)
            nc.sync.dma_start(mcat[:], d["pack"][0:R, _SKEL:_SKEL + MC])

            # one-hot chunks in pass order, halves split across the two
            # HWDGE rings (sync + scalar); pool rotation (bufs=2) gates
            # chunk i+2 on pass i's matmuls.
            H = GB * T // 2
            oh_t = []
            for ci in range(NPASS):
                t_ = ohp.tile([R, GB * T], FP8, tag="oh", name=f"oh_{ci}")
                c0 = ci * GB * T
                nc.sync.dma_start(t_[:, 0:H], d["ohall"][:, c0:c0 + H])
                nc.scalar.dma_start(t_[:, H:2 * H], d["ohall"][:, c0 + H:c0 + 2 * H])
                oh_t.append(t_)

            us_t = P[:, _US:_US + 128]

            # ---- parameter tables (no device transposes) ----
            # cv row first: it feeds the mcat spray DMA (longest dep chain)
            p_cvr = ps.tile([1, VOCAB], F32, tag="pP2", bufs=2)
            nc.tensor.matmul(p_cvr[:], P[0:DV, _WFR:_WFR + 1],
                             P[0:DV, _EVT:_EVT + VOCAB], start=True, stop=True)
            cv_row = sb.tile([1, VOCAB], F16)
            nc.scalar.copy(cv_row[:], p_cvr[:])
            # Ev arrives row-permuted (perm(k) = 10(k%10) + k//10), so the cv
            # row comes out as cv_row[0, 10i+j] = cv[10j+i]; a plain [1,100]
            # -> [10,10] DMA spray then yields mcat[100+i, 32+j] = cv[10j+i].
            nc.scalar.dma_start(mcat[100:110, 32:42], cv_row[0:1, 0:VOCAB])

            # S and cq in one accumulation group: p_s = EqT.T@[Wa|Wfq] + [ba|bf]
            p_s = ps.tile([VOCAB, M + 1], F32, tag="pA", bufs=2)
            nc.tensor.matmul(p_s[:], P[0:DQ, _EQT:_EQT + VOCAB],
                             P[0:DQ, _WAQ:_WAQ + M + 1], start=True, stop=False)
            nc.tensor.matmul(p_s[:], P[0:1, _ONE:_ONE + VOCAB],
                             P[0:1, _BIA:_BIA + M + 1],
                             start=False, stop=True)
            nc.scalar.copy(mcat[0:VOCAB, 52:53], p_s[:, M:M + 1])
            smx = sb.tile([VOCAB, M + 2], F32)
            nc.vector.tensor_reduce(smx[:, M:M + 1], p_s[:, 0:M], AX.X, OP.max)
            nc.vector.tensor_scalar_mul(smx[:, M:M + 1], smx[:, M:M + 1], -1.0)
            nc.scalar.activation(smx[:, 0:M], p_s[:, 0:M],
                                 mybir.ActivationFunctionType.Exp,
                                 bias=smx[:, M:M + 1], scale=1.0)
            nc.vector.tensor_reduce(smx[:, M + 1:M + 2], smx[:, 0:M], AX.X, OP.add)
            nc.vector.reciprocal(smx[:, M + 1:M + 2], smx[:, M + 1:M + 2])
            nc.vector.tensor_scalar(out=mcat[0:VOCAB, 0:M], in0=smx[:, 0:M],
                                    scalar1=smx[:, M + 1:M + 2], scalar2=None,
                                    op0=OP.mult)

            # ---- main pipeline ----
            osl = sb.tile([128, BS], F32)

            for pi in range(NPASS):
                oh_g = oh_t[pi]
                pAs = []
                for half in range(2):
                    pA = ps.tile([128, 1024], F32, tag="pA", name=f"pA_{half}",
                                 bufs=2)
                    for k in range(16):
                        kb = half * 16 + k
                        nc.tensor.matmul(pA[:, k * 64:k * 64 + MC],
                                         oh_g[:, kb * T:(kb + 1) * T],
                                         mcat[:], start=True, stop=True)
                    pAs.append(pA)
                ws = wk.tile([128, WS], F16, tag="ws")
                a_g = ws[:, _A:_A + 1024]
                for half in range(2):
                    pA3 = pAs[half][:].rearrange("p (k c) -> p k c", c=64)
                    # A contiguous; digit/cq block at 22-col (44B) stride
                    nc.scalar.copy(
                        ws[:, _A + half * 512:_A + (half + 1) * 512].rearrange(
                            "p (k c) -> p k c", c=M),
                        pA3[:, :, 0:M])
                    nc.scalar.copy(
                        ws[:, _DGC + half * 352:_DGC + (half + 1) * 352].rearrange(
                            "p (k c) -> p k c", c=22),
                        pA3[:, :, 32:54])
                dg3 = ws[:, _DGC:_DGC + 704].rearrange("p (k c) -> p k c", c=22)
                # cv[w] = sum_j cvt[j] * ind[j]
                nc.vector.tensor_tensor(
                    ws[:, _CVP:_CVP + 320].rearrange("p (k c) -> p k c", c=10),
                    dg3[:, :, 0:10], dg3[:, :, 10:20], OP.mult)
                with nc.allow_low_precision(reason="10-term f16 dot of one-hot"):
                    nc.vector.tensor_reduce(
                        ws[:, _CVW:_CVW + GB],
                        ws[:, _CVP:_CVP + 320].rearrange("p (k c) -> p k c", c=10),
                        AX.X, OP.add)
                # v = A * cv[w] (cv broadcast along m) on GpSimd
                a3 = a_g.rearrange("p (k c) -> p k c", c=M)
                cvb = ws[:, _CVW:_CVW + GB].rearrange("p (k c) -> p k c", c=1)
                a3b, cvb = bass.broadcast_tensor_aps(a3, cvb)
                nc.gpsimd.tensor_tensor(
                    ws[:, _V:_V + 1024].rearrange("p (k c) -> p k c", c=M),
                    a3b, cvb, OP.mult)
                # exclusive cumsum over t (strict upper as lhsT)
                pP = ps.tile([128, 1024], F32, tag="pP2", name="pP", bufs=2)
                for half in range(2):
                    nc.tensor.matmul(pP[:, half * 512:(half + 1) * 512], us_t,
                                     ws[:, _V + half * 512:_V + (half + 1) * 512],
                                     start=True, stop=True)
                # pred contribution terms: A * C (C straight from PSUM)
                nc.vector.tensor_tensor(
                    ws[:, _AP:_AP + 1024], a_g, pP[:], OP.mult)
                with nc.allow_low_precision(reason="32-term f16 dot, tol 2e-2"):
                    nc.vector.tensor_reduce(
                        ws[:, _O16:_O16 + GB],
                        ws[:, _AP:_AP + 1024].rearrange("p (b m) -> p b m", m=M),
                        AX.X, OP.add)
                nc.vector.tensor_add(
                    osl[:, pi * GB:(pi + 1) * GB].rearrange(
                        "p (k c) -> p k c", c=1),
                    ws[:, _O16:_O16 + GB].rearrange("p (k c) -> p k c", c=1),
                    dg3[:, :, 20:21])
                nc.sync.dma_start(preds[:, pi * GB:(pi + 1) * GB],
                                  osl[:, pi * GB:(pi + 1) * GB])

    nc.compile()
    return nc


@functools.lru_cache(maxsize=1)
def _get_nc():
    return _build()


def _in_maps(questions, answers, Eq, Ev, Wa, ba, Wf, bf):
    questions = np.asarray(questions)
    answers = np.asarray(answers)
    w = (questions.astype(np.int64) * 2 + answers.astype(np.int64)) % VOCAB
    pack = np.zeros((128, PC), np.float16)
    pack[0:DQ, _EQT:_EQT + VOCAB] = np.asarray(Eq, np.float32).T
    # Ev rows permuted so the derived cv row is emitted in (i-major) order
    perm = np.array([10 * (k % 10) + k // 10 for k in range(VOCAB)])
    pack[0:DV, _EVT:_EVT + VOCAB] = np.asarray(Ev, np.float32)[perm].T
    wf = np.asarray(Wf, np.float32).reshape(DQ + DV)
    pack[0:DQ, _WAQ:_WAQ + M] = np.asarray(Wa, np.float32)
    pack[0:DQ, _WAQ + M] = wf[0:DQ]
    pack[0:DV, _WFR] = wf[DQ:DQ + DV]
    pack[0, _BIA:_BIA + M] = np.asarray(ba, np.float32).reshape(M)
    pack[0, _BIA + M] = np.asarray(bf, np.float32).reshape(())
    pack[0, _ONE:_ONE + VOCAB] = 1.0
    pack[:, _US:_US + 128] = np.triu(np.ones((128, 128), np.float16), k=1)
    # mcat skeleton: zeros except I10 at rows 110:120, cols 42:52
    skel = np.zeros((128, MC), np.float16)
    skel[110:120, 42:52] = np.eye(10, dtype=np.float16)
    pack[:, _SKEL:_SKEL + MC] = skel
    in_maps = []
    for c in range(NCORES):
        sl = slice(c * BS, (c + 1) * BS)
        qf = np.ascontiguousarray(questions[:, sl].T).ravel()
        wfl = np.ascontiguousarray(w[:, sl].T).ravel()
        oh = np.zeros((R, N), dtype=ml_dtypes.float8_e4m3)
        ar = np.arange(N)
        oh[qf, ar] = 1.0
        oh[100 + wfl % 10, ar] = 1.0
        oh[110 + wfl // 10, ar] = 1.0
        in_maps.append({"pack": pack, "ohall": oh})
    return in_maps


def kernel(questions, answers, Eq, Ev, Wa, ba, Wf, bf):
    nc = _get_nc()
    in_maps = _in_maps(questions, answers, Eq, Ev, Wa, ba, Wf, bf)
    res = run_bass_kernel_spmd(nc, in_maps, list(range(NCORES)))
    preds = np.concatenate([res.results[c]["preds"] for c in range(NCORES)], axis=1)
    return preds.astype(np.float32)
